# revision 1
# baseline (speedup 1.0000x reference)
"""ChoiceAttention Trainium2 kernel.

Math (per batch item b, per "retain" iteration a over the 5 options):
    q_a = opt_a @ W                              (s, h)
    S_ak[p, r] = q_a[p, :] . opt_k[r, :]         for the 4 options k != a
    w_ak = softmax over k of (S_ak + bias)       (bias cancels: softmax is
                                                  shift-invariant over k)
    out += sum_k w_ak @ opt_k
final out /= 2.

Sharding: data-parallel over batch across 8 NeuronCores (4 items each),
W replicated. No collectives; host concatenates the per-core outputs.

Layout strategy per core / batch item:
    nat_k : opt_k natural layout      (128p, 2 sc, 1024h)  - DMA'd in
    x_k   : opt_k transposed (h-major)(128p, 8 hc, 256s)   - PE transposes
    q_a^T : h-major q                 (128p, 8 hc, 256s)   - matmul(W, x_a)
    S_ak^T: scores transposed         (128p, 2 rc, 256p)   - matmul(x_k, q_a^T)
    softmax over the four k tiles elementwise (max-subtract, exp, recip)
    out   : accumulated in 4 PSUM banks over all 40 (a,k,rc) matmul groups
All matmuls run as float32r (full PE rate, fp32 storage).
"""

import numpy as np

B, S, H = 32, 256, 1024
NCORES = 8
BPC = B // NCORES  # batch items per core
P = 128
HC = H // P  # 8 h-chunks
SC = S // P  # 2 s-chunks
NOPT = 5

_CACHE: dict = {}


def _build_bass(reps: int = 1, cfg: dict | None = None):
    cfg = dict(cfg or {})
    NAT_BUFS = cfg.get("nat_bufs", 7)
    XT_BUFS = cfg.get("xt_bufs", NOPT)
    WS_BUFS = cfg.get("ws_bufs", 5)
    E_BUFS = cfg.get("e_bufs", 5)
    OSB_BUFS = cfg.get("osb_bufs", 1)
    GP_SUB = cfg.get("gp_sub", False)
    PSM = cfg.get("ps_misc", 2)
    PSS = cfg.get("ps_s", 2)
    PSO = cfg.get("ps_o", 4)
    from contextlib import ExitStack

    import concourse.mybir as mybir
    import concourse.tile as tile
    from concourse import bacc
    from concourse.masks import make_identity

    FP32 = mybir.dt.float32
    F32R = mybir.dt.float32r
    AF = mybir.ActivationFunctionType

    nc = bacc.Bacc(debug=False)

    opt_d = [
        nc.dram_tensor(f"option{i + 1}", (BPC, S, H), F32R, kind="ExternalInput")
        for i in range(NOPT)
    ]
    w_d = nc.dram_tensor("W", (H, H), F32R, kind="ExternalInput")
    out_d = nc.dram_tensor("out", (BPC, S, H), FP32, kind="ExternalOutput")

    with ExitStack() as ctx:
        tc = ctx.enter_context(tile.TileContext(nc))
        const = ctx.enter_context(tc.tile_pool(name="const", bufs=1))
        natp = ctx.enter_context(tc.tile_pool(name="nat", bufs=NAT_BUFS))
        xp = ctx.enter_context(tc.tile_pool(name="xt", bufs=XT_BUFS))
        qp = ctx.enter_context(tc.tile_pool(name="qq", bufs=3))
        sp = ctx.enter_context(tc.tile_pool(name="ss", bufs=6))
        ep = ctx.enter_context(tc.tile_pool(name="ee", bufs=E_BUFS))
        mp_ = ctx.enter_context(tc.tile_pool(name="mm", bufs=2))
        zp = ctx.enter_context(tc.tile_pool(name="zz", bufs=2))
        rp = ctx.enter_context(tc.tile_pool(name="rr", bufs=2))
        wsp = ctx.enter_context(tc.tile_pool(name="wsum", bufs=WS_BUFS))
        op_ = ctx.enter_context(tc.tile_pool(name="osb", bufs=OSB_BUFS))
        ps_misc = ctx.enter_context(tc.tile_pool(name="ps_misc", bufs=PSM, space="PSUM"))
        ps_s = ctx.enter_context(tc.tile_pool(name="ps_s", bufs=PSS, space="PSUM"))
        ps_o = ctx.enter_context(tc.tile_pool(name="ps_o", bufs=PSO, space="PSUM"))

        ident_f = const.tile([P, P], FP32)
        make_identity(nc, ident_f)
        ident = const.tile([P, P], F32R)
        nc.vector.tensor_copy(out=ident, in_=ident_f)
        w_sb = const.tile([P, HC, H], F32R)
        w_loaded = [False]

        from contextlib import nullcontext

        loop_cm = tc.For_i(0, reps, 1) if reps > 1 else nullcontext()
        with loop_cm:
            # cross-batch carried prefetch of options 0 and 1
            carry = {"nat": {}, "x": {}}

            def load_nat(b, k):
                nk = natp.tile([P, SC, H], F32R, tag="nat", name=f"nat_{b}_{k}")
                nc.sync.dma_start(
                    out=nk, in_=opt_d[k].ap()[b].rearrange("(sc p) h -> p sc h", p=P)
                )
                return nk

            def transpose_opt(b, k, nk):
                xk = xp.tile([P, HC, S], F32R, tag="xt", name=f"x_{b}_{k}")
                for j in range(HC // 2):  # pairs of h-chunks -> one PSUM bank
                    pt = ps_misc.tile([P, 4, P], F32R, tag="ps_misc",
                                      name=f"pt_{b}_{k}_{j}")
                    for d in range(2):
                        hc = 2 * j + d
                        for sc in range(SC):
                            nc.tensor.transpose(
                                out=pt[:, 2 * d + sc, :],
                                in_=nk[:, sc, hc * P : (hc + 1) * P],
                                identity=ident,
                            )
                    dst = xk[:, 2 * j : 2 * j + 2, :]
                    if (k + j) % 2 == 0:
                        nc.scalar.copy(out=dst, in_=pt)
                    else:
                        nc.vector.tensor_copy(out=dst, in_=pt)
                return xk

            for b in range(BPC):
                # ---- load options; 0/1 may be carried from prev tail ----
                nat = []
                for k in range(NOPT):
                    nat.append(carry["nat"].get(k) or load_nat(b, k))
                if b == 0:
                    # W on the ACT hwdge ring so it never blocks option loads
                    nc.scalar.dma_start(
                        out=w_sb, in_=w_d.ap().rearrange("(kc p) h -> p kc h", p=P)
                    )
                x = []
                for k in range(NOPT):
                    x.append(carry["x"].get(k) or transpose_opt(b, k, nat[k]))
                carry["nat"] = {}
                carry["x"] = {}

                # ---- q_a^T = W^T @ opt_a^T, pipelined with the a-loop ----
                q = [None] * NOPT

                def emit_q(a):
                    qt = qp.tile([P, HC, S], F32R, tag="qq", name=f"q_{b}_{a}")
                    for half in range(HC // 2):
                        pq = ps_misc.tile([P, 2, S], FP32, tag="ps_misc",
                                          name=f"pq_{b}_{a}_{half}")
                        for d in range(2):
                            mc = 2 * half + d
                            for kc in range(HC):
                                nc.tensor.matmul(
                                    pq[:, d, :],
                                    w_sb[:, kc, mc * P : (mc + 1) * P],
                                    x[a][:, kc, :],
                                    start=(kc == 0),
                                    stop=(kc == HC - 1),
                                )
                        nc.scalar.copy(out=qt[:, 2 * half : 2 * half + 2, :], in_=pq)
                    q[a] = qt

                def emit_scores(a):
                    s_sb = []
                    for k in range(NOPT):
                        if k == a:
                            continue
                        st = ps_s.tile([P, SC, S], FP32, tag="ps_s",
                                       name=f"st_{b}_{a}_{k}")
                        for rc in range(SC):
                            for hc in range(HC):
                                nc.tensor.matmul(
                                    st[:, rc, :],
                                    x[k][:, hc, rc * P : (rc + 1) * P],
                                    q[a][:, hc, :],
                                    start=(hc == 0),
                                    stop=(hc == HC - 1),
                                )
                        ssb = sp.tile([P, SC, S], FP32, tag="ss",
                                      name=f"ssb_{b}_{a}_{k}")
                        nc.scalar.copy(out=ssb, in_=st)
                        s_sb.append(ssb)
                    return s_sb

                # wsum[k] accumulates sum_a softmax_weight(a, k): the output
                # matmul collapses to sum_k wsum_k @ opt_k (4x fewer matmuls)
                wsum = [None] * NOPT

                def emit_softmax(a, s_sb):
                    m = mp_.tile([P, SC, S], FP32, tag="mm", name=f"m_{b}_{a}")
                    nc.vector.tensor_max(m, s_sb[0], s_sb[1])
                    nc.vector.tensor_max(m, m, s_sb[2])
                    nc.vector.tensor_max(m, m, s_sb[3])
                    e = []
                    for k4 in range(4):
                        sub_eng = nc.gpsimd if GP_SUB else nc.vector
                        sub_eng.tensor_sub(s_sb[k4], s_sb[k4], m)
                        ek = ep.tile([P, SC, S], F32R, tag="ee",
                                     name=f"e_{b}_{a}_{k4}")
                        nc.scalar.activation(out=ek, in_=s_sb[k4], func=AF.Exp)
                        e.append(ek)
                    z = zp.tile([P, SC, S], FP32, tag="zz", name=f"z_{b}_{a}")
                    rcp = rp.tile([P, SC, S], FP32, tag="rr", name=f"r_{b}_{a}")
                    nc.vector.tensor_add(z, e[0], e[1])
                    nc.vector.tensor_add(rcp, e[2], e[3])
                    nc.vector.tensor_add(z, z, rcp)
                    nc.vector.reciprocal(rcp, z)
                    ks = [k for k in range(NOPT) if k != a]
                    for k4, k in enumerate(ks):
                        if wsum[k] is None:
                            wk = wsp.tile([P, SC, S], F32R, tag="wsum",
                                          name=f"ws_{b}_{k}")
                            nc.vector.tensor_mul(wk, e[k4], rcp)
                            wsum[k] = wk
                        else:
                            nc.vector.tensor_mul(e[k4], e[k4], rcp)
                            nc.vector.tensor_add(wsum[k], wsum[k], e[k4])

                po = {}
                po_started = {}

                def emit_out_k(k, nn, last):
                    for mp2 in range(SC):
                        key = (mp2, nn)
                        if key not in po:
                            po[key] = ps_o.tile([P, 512], FP32, tag="ps_o",
                                                name=f"po_{b}_{mp2}_{nn}")
                            po_started[key] = False
                        for rc in range(SC):
                            is_last = last and rc == SC - 1
                            nc.tensor.matmul(
                                po[key],
                                wsum[k][:, rc, mp2 * P : (mp2 + 1) * P],
                                nat[k][:, rc, nn * 512 : (nn + 1) * 512],
                                start=(not po_started[key]),
                                stop=is_last,
                            )
                            po_started[key] = True

                emit_q(0)
                emit_q(1)
                s_cur = emit_scores(0)
                for a in range(NOPT):
                    if a + 2 < NOPT:
                        emit_q(a + 2)
                    emit_softmax(a, s_cur)
                    if a + 1 < NOPT:
                        s_cur = emit_scores(a + 1)
                    if a == 1 and b + 1 < BPC:
                        # prefetch next batch's first options (spare nat slots)
                        carry["nat"][0] = load_nat(b + 1, 0)
                    if a == NOPT - 2:
                        if b + 1 < BPC:
                            carry["nat"][1] = load_nat(b + 1, 1)
                            # cover softmax(3)'s tail with next-batch work
                            carry["x"][0] = transpose_opt(
                                b + 1, 0, carry["nat"][0])
                        # wsum for the last option is complete (it never
                        # scores against itself): overlap its out-matmuls
                        # with the final softmax
                        emit_out_k(NOPT - 1, 0, last=False)
                        emit_out_k(NOPT - 1, 1, last=False)
                if b + 1 < BPC:
                    # cover softmax(4)'s tail too
                    carry["x"][1] = transpose_opt(b + 1, 1, carry["nat"][1])
                osb = op_.tile([P, SC, H], FP32, tag="osb", name=f"osb_{b}")
                for k in range(NOPT - 1):
                    last = k == NOPT - 2
                    emit_out_k(k, 0, last=last)
                    emit_out_k(k, 1, last=last)
                for mp2 in range(SC):
                    for nn in range(2):
                        nc.scalar.activation(
                            out=osb[:, mp2, nn * 512 : (nn + 1) * 512],
                            in_=po[(mp2, nn)],
                            func=AF.Copy,
                            scale=0.5,
                        )
                nc.scalar.dma_start(
                    out=out_d.ap()[b].rearrange("(sc p) h -> p sc h", p=P), in_=osb
                )

    nc.compile()
    return nc


def _get_nc(reps: int = 1, cfg: dict | None = None):
    key = f"nc{reps}-{sorted((cfg or {}).items())}"
    if key not in _CACHE:
        _CACHE[key] = _build_bass(reps, cfg)
    return _CACHE[key]


def kernel(**inputs) -> np.ndarray:
    from concourse.bass_utils import run_bass_kernel_spmd

    nc = _get_nc()
    opts = [np.ascontiguousarray(np.asarray(inputs[f"option{i + 1}"], dtype=np.float32))
            for i in range(NOPT)]
    W = np.ascontiguousarray(np.asarray(inputs["W"], dtype=np.float32))

    in_maps = []
    for c in range(NCORES):
        m = {f"option{i + 1}": opts[i][c * BPC : (c + 1) * BPC] for i in range(NOPT)}
        m["W"] = W
        in_maps.append(m)

    res = run_bass_kernel_spmd(nc, in_maps, list(range(NCORES)))
    out = np.concatenate([res.results[c]["out"] for c in range(NCORES)], axis=0)
    return np.asarray(out, dtype=np.float32)



# revision 14
# speedup vs baseline: 1.5485x; 1.5485x over previous
"""ChoiceAttention Trainium2 kernel.

Math (per batch item b, per "retain" iteration a over the 5 options):
    q_a = opt_a @ W                              (s, h)
    S_ak[p, r] = q_a[p, :] . opt_k[r, :]         for the 4 options k != a
    w_ak = softmax over k of (S_ak + bias)       (bias cancels: softmax is
                                                  shift-invariant over k)
    out += sum_k w_ak @ opt_k
final out /= 2.

Sharding: data-parallel over batch across 8 NeuronCores (4 items each),
W replicated. No collectives; host concatenates the per-core outputs.

Layout strategy per core / batch item:
    nat_k : opt_k natural layout      (128p, 2 sc, 1024h)  - DMA'd in
    x_k   : opt_k transposed (h-major)(128p, 8 hc, 256s)   - PE transposes
    q_a^T : h-major q                 (128p, 8 hc, 256s)   - matmul(W, x_a)
    S_ak^T: scores transposed         (128p, 2 rc, 256p)   - matmul(x_k, q_a^T)
    softmax over the four k tiles elementwise (max-subtract, exp, recip)
    out   : accumulated in 4 PSUM banks over all 40 (a,k,rc) matmul groups
All matmuls run as float32r (full PE rate, fp32 storage).
"""

import numpy as np

B, S, H = 32, 256, 1024
NCORES = 8
BPC = B // NCORES  # batch items per core
P = 128
HC = H // P  # 8 h-chunks
SC = S // P  # 2 s-chunks
NOPT = 5

_CACHE: dict = {}


def _build_bass(reps: int = 1, cfg: dict | None = None):
    cfg = dict(cfg or {})
    NAT_BUFS = cfg.get("nat_bufs", 7)
    XT_BUFS = cfg.get("xt_bufs", NOPT)
    WS_BUFS = cfg.get("ws_bufs", 5)
    E_BUFS = cfg.get("e_bufs", 5)
    OSB_BUFS = cfg.get("osb_bufs", 1)
    GP_SUB = cfg.get("gp_sub", False)
    PSM = cfg.get("ps_misc", 2)
    PSS = cfg.get("ps_s", 2)
    PSO = cfg.get("ps_o", 4)
    from contextlib import ExitStack

    import concourse.mybir as mybir
    import concourse.tile as tile
    from concourse import bacc
    from concourse.masks import make_identity

    FP32 = mybir.dt.float32
    F32R = mybir.dt.float32r
    BF16 = mybir.dt.bfloat16
    AF = mybir.ActivationFunctionType

    nc = bacc.Bacc(debug=False)

    opt_d = [
        nc.dram_tensor(f"option{i + 1}", (BPC, S, H), F32R, kind="ExternalInput")
        for i in range(NOPT)
    ]
    w_d = nc.dram_tensor("W", (H, H), F32R, kind="ExternalInput")
    out_d = nc.dram_tensor("out", (BPC, S, H), FP32, kind="ExternalOutput")

    with ExitStack() as ctx:
        tc = ctx.enter_context(tile.TileContext(nc))
        const = ctx.enter_context(tc.tile_pool(name="const", bufs=1))
        natp = ctx.enter_context(tc.tile_pool(name="natf", bufs=3))
        natbf = ctx.enter_context(tc.tile_pool(name="nat", bufs=NAT_BUFS))
        xp = ctx.enter_context(tc.tile_pool(name="xt", bufs=XT_BUFS))
        qp = ctx.enter_context(tc.tile_pool(name="qq", bufs=3))
        sp = ctx.enter_context(tc.tile_pool(name="ss", bufs=6))
        ep = ctx.enter_context(tc.tile_pool(name="ee", bufs=E_BUFS))
        mp_ = ctx.enter_context(tc.tile_pool(name="mm", bufs=2))
        zp = ctx.enter_context(tc.tile_pool(name="zz", bufs=2))
        rp = ctx.enter_context(tc.tile_pool(name="rr", bufs=2))
        wsp = ctx.enter_context(tc.tile_pool(name="wsum", bufs=WS_BUFS))
        wbfp = ctx.enter_context(tc.tile_pool(name="wbf", bufs=NOPT))
        op_ = ctx.enter_context(tc.tile_pool(name="osb", bufs=OSB_BUFS))
        ps_misc = ctx.enter_context(tc.tile_pool(name="ps_misc", bufs=PSM, space="PSUM"))
        ps_s = ctx.enter_context(tc.tile_pool(name="ps_s", bufs=PSS, space="PSUM"))
        ps_o = ctx.enter_context(tc.tile_pool(name="ps_o", bufs=PSO, space="PSUM"))

        ident_f = const.tile([P, P], FP32)
        make_identity(nc, ident_f)
        ident = const.tile([P, P], BF16)
        nc.vector.tensor_copy(out=ident, in_=ident_f)
        w_sb = const.tile([P, HC, H], BF16)

        from contextlib import nullcontext

        loop_cm = tc.For_i(0, reps, 1) if reps > 1 else nullcontext()
        with loop_cm:
            # cross-batch carried prefetch of options 0 and 1
            carry = {"nat": {}, "x": {}}

            def load_nat(b, k):
                nf = natp.tile([P, SC, H], F32R, tag="natf", name=f"natf_{b}_{k}")
                nc.sync.dma_start(
                    out=nf, in_=opt_d[k].ap()[b].rearrange("(sc p) h -> p sc h", p=P)
                )
                nk = natbf.tile([P, SC, H], BF16, tag="nat", name=f"nat_{b}_{k}")
                nc.gpsimd.tensor_copy(out=nk, in_=nf)
                return nk

            def transpose_opt(b, k, nk):
                xk = xp.tile([P, HC, S], BF16, tag="xt", name=f"x_{b}_{k}")
                for j in range(HC // 2):  # pairs of h-chunks -> one PSUM bank
                    pt = ps_misc.tile([P, 4, P], BF16, tag="ps_misc",
                                      name=f"pt_{b}_{k}_{j}")
                    for d in range(2):
                        hc = 2 * j + d
                        for sc in range(SC):
                            nc.tensor.transpose(
                                out=pt[:, 2 * d + sc, :],
                                in_=nk[:, sc, hc * P : (hc + 1) * P],
                                identity=ident,
                            )
                    dst = xk[:, 2 * j : 2 * j + 2, :]
                    if (k + j) % 2 == 0:
                        nc.scalar.copy(out=dst, in_=pt)
                    else:
                        nc.vector.tensor_copy(out=dst, in_=pt)
                return xk

            for b in range(BPC):
                # ---- load options; 0/1 may be carried from prev tail ----
                nat = []
                for k in range(NOPT):
                    nat.append(carry["nat"].get(k) or load_nat(b, k))
                if b == 0:
                    # stage W through the fp32 nat pool in quarters, convert
                    # to bf16 (scalar ring so it never blocks option loads)
                    for quarter in range(4):
                        wst = natp.tile([P, SC, H], F32R, tag="natf",
                                        name=f"wst_{quarter}")
                        nc.scalar.dma_start(
                            out=wst,
                            in_=w_d.ap()
                            .rearrange("(kc p) h -> p kc h", p=P)[
                                :, 2 * quarter : 2 * quarter + 2, :
                            ],
                        )
                        nc.gpsimd.tensor_copy(
                            out=w_sb[:, 2 * quarter : 2 * quarter + 2, :], in_=wst
                        )
                x = []
                for k in range(NOPT):
                    x.append(carry["x"].get(k) or transpose_opt(b, k, nat[k]))
                carry["nat"] = {}
                carry["x"] = {}

                # ---- q_a^T = W^T @ opt_a^T, pipelined with the a-loop ----
                q = [None] * NOPT

                def emit_q(a):
                    qt = qp.tile([P, HC, S], BF16, tag="qq", name=f"q_{b}_{a}")
                    for half in range(HC // 2):
                        pq = ps_misc.tile([P, 2, S], FP32, tag="ps_misc",
                                          name=f"pq_{b}_{a}_{half}")
                        for d in range(2):
                            mc = 2 * half + d
                            for kc in range(HC):
                                nc.tensor.matmul(
                                    pq[:, d, :],
                                    w_sb[:, kc, mc * P : (mc + 1) * P],
                                    x[a][:, kc, :],
                                    start=(kc == 0),
                                    stop=(kc == HC - 1),
                                )
                        nc.scalar.copy(out=qt[:, 2 * half : 2 * half + 2, :], in_=pq)
                    q[a] = qt

                def emit_scores(a):
                    s_sb = []
                    for k in range(NOPT):
                        if k == a:
                            continue
                        st = ps_s.tile([P, SC, S], FP32, tag="ps_s",
                                       name=f"st_{b}_{a}_{k}")
                        for rc in range(SC):
                            for hc in range(HC):
                                nc.tensor.matmul(
                                    st[:, rc, :],
                                    x[k][:, hc, rc * P : (rc + 1) * P],
                                    q[a][:, hc, :],
                                    start=(hc == 0),
                                    stop=(hc == HC - 1),
                                )
                        ssb = sp.tile([P, SC, S], FP32, tag="ss",
                                      name=f"ssb_{b}_{a}_{k}")
                        nc.scalar.copy(out=ssb, in_=st)
                        s_sb.append(ssb)
                    return s_sb

                # wsum[k] accumulates sum_a softmax_weight(a, k): the output
                # matmul collapses to sum_k wsum_k @ opt_k (4x fewer matmuls)
                wsum = [None] * NOPT

                def emit_softmax(a, s_sb):
                    m = mp_.tile([P, SC, S], FP32, tag="mm", name=f"m_{b}_{a}")
                    nc.vector.tensor_max(m, s_sb[0], s_sb[1])
                    nc.vector.tensor_max(m, m, s_sb[2])
                    nc.vector.tensor_max(m, m, s_sb[3])
                    e = []
                    for k4 in range(4):
                        sub_eng = nc.gpsimd if GP_SUB else nc.vector
                        sub_eng.tensor_sub(s_sb[k4], s_sb[k4], m)
                        ek = ep.tile([P, SC, S], F32R, tag="ee",
                                     name=f"e_{b}_{a}_{k4}")
                        nc.scalar.activation(out=ek, in_=s_sb[k4], func=AF.Exp)
                        e.append(ek)
                    z = zp.tile([P, SC, S], FP32, tag="zz", name=f"z_{b}_{a}")
                    rcp = rp.tile([P, SC, S], FP32, tag="rr", name=f"r_{b}_{a}")
                    nc.vector.tensor_add(z, e[0], e[1])
                    nc.vector.tensor_add(rcp, e[2], e[3])
                    nc.vector.tensor_add(z, z, rcp)
                    nc.vector.reciprocal(rcp, z)
                    ks = [k for k in range(NOPT) if k != a]
                    for k4, k in enumerate(ks):
                        if wsum[k] is None:
                            wk = wsp.tile([P, SC, S], F32R, tag="wsum",
                                          name=f"ws_{b}_{k}")
                            nc.vector.tensor_mul(wk, e[k4], rcp)
                            wsum[k] = wk
                        else:
                            nc.vector.tensor_mul(e[k4], e[k4], rcp)
                            nc.vector.tensor_add(wsum[k], wsum[k], e[k4])

                po = {}
                po_started = {}
                wbf = [None] * NOPT

                def conv_wsum(k):
                    wb = wbfp.tile([P, SC, S], BF16, tag="wbf", name=f"wb_{b}_{k}")
                    nc.gpsimd.tensor_copy(out=wb, in_=wsum[k])
                    wbf[k] = wb

                def emit_out_k(k, nn, last):
                    for mp2 in range(SC):
                        key = (mp2, nn)
                        if key not in po:
                            po[key] = ps_o.tile([P, 512], FP32, tag="ps_o",
                                                name=f"po_{b}_{mp2}_{nn}")
                            po_started[key] = False
                        for rc in range(SC):
                            is_last = last and rc == SC - 1
                            nc.tensor.matmul(
                                po[key],
                                wbf[k][:, rc, mp2 * P : (mp2 + 1) * P],
                                nat[k][:, rc, nn * 512 : (nn + 1) * 512],
                                start=(not po_started[key]),
                                stop=is_last,
                            )
                            po_started[key] = True

                emit_q(0)
                emit_q(1)
                s_cur = emit_scores(0)
                for a in range(NOPT):
                    if a + 2 < NOPT:
                        emit_q(a + 2)
                    emit_softmax(a, s_cur)
                    if a + 1 < NOPT:
                        s_cur = emit_scores(a + 1)
                    if a == 1 and b + 1 < BPC:
                        # prefetch next batch's first options (spare nat slots)
                        carry["nat"][0] = load_nat(b + 1, 0)
                    if a == NOPT - 2:
                        if b + 1 < BPC:
                            carry["nat"][1] = load_nat(b + 1, 1)
                            # cover softmax(3)'s tail with next-batch work
                            carry["x"][0] = transpose_opt(
                                b + 1, 0, carry["nat"][0])
                        # wsum for the last option is complete (it never
                        # scores against itself): overlap its out-matmuls
                        # with the final softmax
                        conv_wsum(NOPT - 1)
                        emit_out_k(NOPT - 1, 0, last=False)
                        emit_out_k(NOPT - 1, 1, last=False)
                if b + 1 < BPC:
                    # cover softmax(4)'s tail too
                    carry["x"][1] = transpose_opt(b + 1, 1, carry["nat"][1])
                osb = op_.tile([P, SC, H], FP32, tag="osb", name=f"osb_{b}")
                for k in range(NOPT - 1):
                    last = k == NOPT - 2
                    conv_wsum(k)
                    emit_out_k(k, 0, last=last)
                    emit_out_k(k, 1, last=last)
                for mp2 in range(SC):
                    for nn in range(2):
                        nc.scalar.activation(
                            out=osb[:, mp2, nn * 512 : (nn + 1) * 512],
                            in_=po[(mp2, nn)],
                            func=AF.Copy,
                            scale=0.5,
                        )
                nc.scalar.dma_start(
                    out=out_d.ap()[b].rearrange("(sc p) h -> p sc h", p=P), in_=osb
                )

    nc.compile()
    return nc


def _get_nc(reps: int = 1, cfg: dict | None = None):
    key = f"nc{reps}-{sorted((cfg or {}).items())}"
    if key not in _CACHE:
        _CACHE[key] = _build_bass(reps, cfg)
    return _CACHE[key]


def kernel(**inputs) -> np.ndarray:
    from concourse.bass_utils import run_bass_kernel_spmd

    nc = _get_nc()
    opts = [np.ascontiguousarray(np.asarray(inputs[f"option{i + 1}"], dtype=np.float32))
            for i in range(NOPT)]
    W = np.ascontiguousarray(np.asarray(inputs["W"], dtype=np.float32))

    in_maps = []
    for c in range(NCORES):
        m = {f"option{i + 1}": opts[i][c * BPC : (c + 1) * BPC] for i in range(NOPT)}
        m["W"] = W
        in_maps.append(m)

    res = run_bass_kernel_spmd(nc, in_maps, list(range(NCORES)))
    out = np.concatenate([res.results[c]["out"] for c in range(NCORES)], axis=0)
    return np.asarray(out, dtype=np.float32)

